# revision 10
# baseline (speedup 1.0000x reference)
"""Distributed multi-head attention for 8 trn2 NeuronCores.

Strategy (Ulysses-style head-sharding), v3 schedule:
  - Every core receives the full activations pre-transposed/cast host-side:
    xT [C, B*N] bf16. Heads are sharded 2-per-core for QKV + attention
    (Megatron column-sharded QKV weights). Scores are computed TRANSPOSED
    (keys on partitions, queries on free) so the softmaxed probabilities
    feed the P@V matmul directly; the softmax denominator comes from a
    ones-column appended to V.
  - The softmax exp stream on the Scalar engine (~139us floor) is the
    pacing resource, so the Scalar engine runs NOTHING else: V is
    produced directly in keys-on-partitions layout by matmuls with the
    xT block as the stationary operand (no DMA transposes anywhere), and
    all other DMA triggers live on the sync/gpsimd queues.
  - Attention for batch 0 starts after only K(b0)+Q(b0,rc0)+V(rows
    0..1024); every remaining QKV group is injected as a PE filler
    inside the attention loop. V-group PSUM interleaves with the
    attention-output accumulator in the same 2-slot pool ring (one V
    group per query chunk), so score tiles never contend with fillers
    for their own PSUM ring beyond a ~0.4us bubble per filler.
  - The qc-boundary normalize (reciprocal of the ones-column denominator,
    partition-broadcast by a K=1 ones-matmul into the accumulator's
    unused partitions 64..127) is deferred past the next chunk's first
    two score matmuls so the PE queue never blocks the exp stream.
  - Per-batch AllToAlls (bf16) re-shard the attention output from
    head-sharded to row-sharded; gathers carry an explicit sync dep on
    their collective and are scheduled ~2 query chunks after it was
    issued, proj one slot after the gather. Only the final 512-row A2A
    + one 128-row proj chunk are exposed as tail.

Walrus constraint: a fused matmul carries at most ONE semaphore wait; an
explicit ldweights before each accumulation-group start gives
move_matmul_waits_to_ldweights a place to park extra waits.
"""

import sys

for _p in ("/opt/trn_rl_repo", "/opt/pypackages"):
    if _p not in sys.path:
        sys.path.append(_p)

import numpy as np
import ml_dtypes

import concourse.bass as bass
import concourse.mybir as mybir
import concourse.tile as tile
from concourse import bacc
from concourse.bass_utils import run_bass_kernel_spmd

P = 128
CORES = 8
B, N, C = 2, 2048, 1024
H, D = 16, 64
R = B * N          # 4096 total rows
HL = H // CORES    # 2 heads per core
DL = HL * D        # 128 head dims per core
RO = R // CORES    # 512 output rows per core
RB = RO // B       # 256 rows per (core, batch)
NKC = N // P       # 16 key chunks of 128 per batch
NQC = N // 512     # 4 query chunks of 512 per batch
CK = C // P        # 8 contraction chunks of 128
SCALE = D ** -0.5  # 0.125

F32 = mybir.dt.float32
BF16 = mybir.dt.bfloat16

# A2A chunking per batch: entries (start_row, n_rows) within the batch.
A2A_CHUNKS = (
    ((0, N // 2), (N // 2, N // 2)),
    ((0, N // 2), (N // 2, N // 4), (3 * N // 4, N // 4)),
)


def build_nc():
    nc = bacc.Bacc("TRN2", target_bir_lowering=False, debug=False,
                   num_devices=CORES)

    xT_d = nc.declare_dram_parameter("xT", [C, R], BF16, isOutput=False)
    wq_d = nc.declare_dram_parameter("wqT", [C, DL], BF16, isOutput=False)
    wk_d = nc.declare_dram_parameter("wkT", [C, DL], BF16, isOutput=False)
    wv_d = nc.declare_dram_parameter("wvT", [C, DL], BF16, isOutput=False)
    wp_d = nc.declare_dram_parameter("wpT", [C, C], BF16, isOutput=False)
    bp_d = nc.declare_dram_parameter("bproj", [C], F32, isOutput=False)
    out_d = nc.declare_dram_parameter("out", [RO, C], F32, isOutput=True)

    with tile.TileContext(nc) as tc:
        build_kernel(tc, xT_d, wq_d, wk_d, wv_d, wp_d, bp_d, out_d)

    nc.compile()
    return nc


def build_kernel(tc, xT_d, wq_d, wk_d, wv_d, wp_d, bp_d, out_d):
    nc = tc.nc
    EXP = mybir.ActivationFunctionType.Exp

    with (
        tc.tile_pool(name="persist", bufs=1) as persist,
        tc.tile_pool(name="expp", bufs=6) as expp,
        tc.tile_pool(name="small", bufs=2) as small,
        tc.tile_pool(name="ypool", bufs=3) as ypool,
        tc.tile_pool(name="pA", bufs=2, space="PSUM") as pA,
        tc.tile_pool(name="pB", bufs=2, space="PSUM") as pB,
        tc.tile_pool(name="dramp", bufs=1, space="DRAM") as dramp,
    ):
        # ---------------- persistent SBUF tensors ----------------
        xT_sb = persist.tile([P, CK, R], BF16, name="xT_sb")
        wq_sb = persist.tile([P, CK, DL], BF16, name="wq_sb")
        wk_sb = persist.tile([P, CK, DL], BF16, name="wk_sb")
        wv_sb = persist.tile([P, CK, DL], BF16, name="wv_sb")
        wp_sb = persist.tile([P, CK, C], BF16, name="wp_sb")
        bias_sb = persist.tile([P, C], F32, name="bias_sb")
        qT_sb = persist.tile([P, R], BF16, name="qT_sb")
        kT_sb = persist.tile([P, R], BF16, name="kT_sb")
        # vaug[:, idx(b,h,kc), :]: [keys=128, D+1]; col D holds ones
        vaug_sb = persist.tile([P, B * HL * NKC, D + 1], BF16, name="vaug_sb")
        oT0_sb = persist.tile([D, R], BF16, name="oT0_sb")
        oT1_sb = persist.tile([D, R], BF16, name="oT1_sb")
        # gathered layout per (b): [c_in_part, src_core, RB rows]
        oTg_sb = persist.tile([P, B, CORES, RB], BF16, name="oTg_sb")
        warm_sb = persist.tile([P, 512], BF16, name="warm_sb")
        ones_sb = persist.tile([1, D], BF16, name="ones_sb")

        a2a_in = {}
        a2a_out = {}
        cc_ops = {}
        for b in range(B):
            for h2, (st0, nr) in enumerate(A2A_CHUNKS[b]):
                blk = nr // CORES
                a2a_in[(b, h2)] = dramp.tile([CORES, DL, blk], BF16,
                                             name=f"a2a_in{b}_{h2}")
                a2a_out[(b, h2)] = dramp.tile([CORES, DL, blk], BF16,
                                              name=f"a2a_out{b}_{h2}")

        def vidx(b, h, kc):
            return (b * HL + h) * NKC + kc

        # ---------------- constants / input DMAs ----------------
        nc.vector.memset(vaug_sb[:, :, D], 1.0)
        nc.vector.memset(warm_sb, 0.0)
        nc.vector.memset(ones_sb, 1.0)

        # PE warmup while input DMAs land: back-to-back matmuls push the
        # HAM clock gate to 8/8 before real work starts
        wps = pA.tile([P, 1024], F32, tag="big", name="wps")
        for i in range(12):
            nc.tensor.matmul(wps[:, 0:512], lhsT=warm_sb[:, 0:128],
                             rhs=warm_sb, start=(i == 0), stop=(i == 11))

        # qkv weights on the scalar queue (before any EXP); xT + Wproj on
        # sync; bias + output writes on gpsimd
        nc.scalar.dma_start(out=wq_sb,
                            in_=wq_d.ap().rearrange("(o p) d -> p o d", p=P))
        nc.scalar.dma_start(out=wk_sb,
                            in_=wk_d.ap().rearrange("(o p) d -> p o d", p=P))
        nc.scalar.dma_start(out=wv_sb,
                            in_=wv_d.ap().rearrange("(o p) d -> p o d", p=P))

        xT_ap = xT_d.ap().rearrange("(o p) n -> p o n", p=P)
        for rc in range(R // 512):
            sl = slice(rc * 512, (rc + 1) * 512)
            nc.sync.dma_start(out=xT_sb[:, :, sl], in_=xT_ap[:, :, sl])
        nc.sync.dma_start(out=wp_sb,
                          in_=wp_d.ap().rearrange("(o p) c -> p o c", p=P))

        bias_bcast = bass.AP(tensor=bp_d, offset=0, ap=[[0, P], [1, C]])
        nc.gpsimd.dma_start(out=bias_sb, in_=bias_bcast)

        # ---------------- QKV building blocks ----------------
        def qkv_group(rc, w_sb, dst):
            sl = slice(rc * 512, (rc + 1) * 512)
            ps = pA.tile([P, 1024], F32, tag="big", name="ps")
            for o in range(CK):
                if o == 0:
                    nc.tensor.ldweights(w_sb[:, o])
                nc.tensor.matmul(ps[:, 0:512], lhsT=w_sb[:, o],
                                 rhs=xT_sb[:, o, sl],
                                 start=(o == 0), stop=(o == CK - 1))
            nc.vector.tensor_copy(out=dst[:, sl], in_=ps[:, 0:512])

        def v_group(g):
            # V rows g*1024..(g+1)*1024 directly in keys-on-partitions
            # layout: out[128 rows, 128 dims] = xT_blk.T @ wv_chunk.
            # Interleaves with otF in the pB ring (one per query chunk).
            b2, half = divmod(g, 2)
            psV = pB.tile([P, 1024], F32, tag="ot", name="psV")
            for j in range(8):
                rsl = slice(g * 1024 + j * P, g * 1024 + (j + 1) * P)
                for o in range(CK):
                    if o == 0:
                        nc.tensor.ldweights(xT_sb[:, o, rsl])
                    nc.tensor.matmul(psV[:, j * P:(j + 1) * P],
                                     lhsT=xT_sb[:, o, rsl],
                                     rhs=wv_sb[:, o],
                                     start=(o == 0), stop=(o == CK - 1))
            for j in range(8):
                kc_b = half * 8 + j
                for h in range(HL):
                    nc.vector.tensor_copy(
                        out=vaug_sb[:, vidx(b2, h, kc_b), 0:D],
                        in_=psV[:, j * P + h * D:j * P + (h + 1) * D])

        # upfront (gates first exp): K(b0) all, Q(b0,rc0), V rows 0..1024
        for rc in range(NQC):
            qkv_group(rc, wk_sb, kT_sb)
        qkv_group(0, wq_sb, qT_sb)
        v_group(0)

        # everything else becomes attention-loop fillers, ordered by when
        # the attention pipeline first needs the result
        fillers = []

        def qkv_filler(rc, w_sb, dst):
            return lambda: qkv_group(rc, w_sb, dst)

        fillers.append(lambda: v_group(1))
        for rc in (1, 2, 3):
            fillers.append(qkv_filler(rc, wq_sb, qT_sb))
        for rc in (4, 5, 6, 7):
            fillers.append(qkv_filler(rc, wk_sb, kT_sb))
        fillers.append(qkv_filler(4, wq_sb, qT_sb))
        fillers.append(lambda: v_group(2))
        fillers.append(lambda: v_group(3))
        for rc in (5, 6, 7):
            fillers.append(qkv_filler(rc, wq_sb, qT_sb))

        # ---------------- A2A + proj building blocks ----------------
        def issue_a2a(b, half):
            st0, nr = A2A_CHUNKS[b][half]
            blk = nr // CORES
            base = b * N + st0
            span = CORES * blk
            src0 = oT0_sb[:, base:base + span].rearrange(
                "d (j r) -> d j r", j=CORES)
            src1 = oT1_sb[:, base:base + span].rearrange(
                "d (j r) -> d j r", j=CORES)
            nc.sync.dma_start(
                out=a2a_in[(b, half)][:, 0:D, :].rearrange("j d r -> d j r"),
                in_=src0)
            nc.sync.dma_start(
                out=a2a_in[(b, half)][:, D:2 * D, :].rearrange("j d r -> d j r"),
                in_=src1)
            cc_ops[(b, half)] = nc.gpsimd.collective_compute(
                "AllToAll",
                mybir.AluOpType.bypass,
                replica_groups=[list(range(CORES))],
                ins=[a2a_in[(b, half)].opt()],
                outs=[a2a_out[(b, half)].opt()],
            )

        def gather_a2a(b, half):
            st0, nr = A2A_CHUNKS[b][half]
            blk = nr // CORES
            boff = st0 // CORES
            d = nc.sync.dma_start(
                out=oTg_sb[:, b, :, boff:boff + blk],
                in_=a2a_out[(b, half)].rearrange("k p r -> p k r"))
            tile.add_dep_helper(d.ins, cc_ops[(b, half)].ins, sync=True,
                                reason="gather after a2a")

        def proj_rows(b, r0, nrows):
            for rr in range(r0, r0 + nrows, P):
                rw = min(P, r0 + nrows - rr)
                rsl = slice(rr, rr + rw)
                for oc in range(C // 512):
                    osl = slice(oc * 512, (oc + 1) * 512)
                    psy = pA.tile([P, 1024], F32, tag="big", name="psy")
                    for o in range(CK):
                        if o == 0:
                            nc.tensor.ldweights(oTg_sb[:, b, o, rsl])
                        nc.tensor.matmul(
                            psy[0:rw, 0:512],
                            lhsT=oTg_sb[:, b, o, rsl],
                            rhs=wp_sb[:, o, osl],
                            start=(o == 0), stop=(o == CK - 1))
                    y_sb = ypool.tile([P, 512], F32, tag="y", name="y_sb")
                    nc.vector.tensor_add(out=y_sb[0:rw],
                                         in0=psy[0:rw, 0:512],
                                         in1=bias_sb[0:rw, osl])
                    nc.gpsimd.dma_start(
                        out=out_d.ap()[b * RB + rr:b * RB + rr + rw, osl],
                        in_=y_sb[0:rw])

        # gather/proj injected at fixed (b,qc) slots ~2 query chunks after
        # their A2A was issued; gather one slot before its proj
        sched = {
            (0, 3): [lambda: gather_a2a(0, 0),
                     lambda: proj_rows(0, 0, RB // 2)],
            (1, 1): [lambda: gather_a2a(0, 1),
                     lambda: proj_rows(0, RB // 2, RB // 2)],
            (1, 3): [lambda: gather_a2a(1, 0),
                     lambda: proj_rows(1, 0, RB // 2)],
        }

        # ---------------- attention: scores/exp/PV pipeline ----------------
        def normalize_and_a2a(qc, b, otF, qsl):
            def run():
                denom = small.tile([1, 1024], F32, tag="denom", name="denom")
                nc.vector.tensor_copy(out=denom, in_=otF[D:D + 1, :])
                recip = small.tile([1, 1024], F32, tag="recip", name="recip")
                nc.vector.reciprocal_approx_fast(out=recip, in_=denom)
                recb = small.tile([1, 1024], BF16, tag="recb", name="recb")
                nc.vector.tensor_copy(out=recb, in_=recip)
                nc.tensor.matmul(otF[D:P, 0:512], lhsT=ones_sb,
                                 rhs=recb[:, 0:512], start=True, stop=True)
                nc.tensor.matmul(otF[D:P, 512:1024], lhsT=ones_sb,
                                 rhs=recb[:, 512:1024], start=True, stop=True)
                rbc = expp.tile([D, 1024], F32, tag="rbc", name="rbc")
                nc.vector.tensor_copy(out=rbc, in_=otF[D:P, :])
                nc.vector.tensor_mul(out=oT0_sb[:, qsl], in0=otF[0:D, 0:512],
                                     in1=rbc[:, 0:512])
                nc.vector.tensor_mul(out=oT1_sb[:, qsl], in0=otF[0:D, 512:],
                                     in1=rbc[:, 512:])
                done_rows = (qc + 1) * 512
                for h2, (st0, nr) in enumerate(A2A_CHUNKS[b]):
                    if st0 + nr == done_rows:
                        issue_a2a(b, h2)
            return run

        for b in range(B):
            for qc in range(NQC):
                pending = list(sched.get((b, qc), []))
                qsl = slice(b * N + qc * 512, b * N + (qc + 1) * 512)
                otF = pB.tile([P, 1024], F32, tag="ot", name="otF")

                def s_pair(kc, qsl=qsl, b=b):
                    kst = b * N + kc * P
                    stF = pA.tile([P, 1024], F32, tag="big", name="stF")
                    for h in range(HL):
                        hsl = slice(h * D, (h + 1) * D)
                        if kc == 0:
                            nc.tensor.ldweights(kT_sb[hsl, kst:kst + P],
                                                tile_position=(h * D, 0))
                        nc.tensor.matmul(stF[:, h * 512:(h + 1) * 512],
                                         lhsT=kT_sb[hsl, kst:kst + P],
                                         rhs=qT_sb[hsl, qsl],
                                         start=True, stop=True)
                    return stF

                def exp_pv(kc, stF, otF=otF, b=b):
                    exF = expp.tile([P, 1024], BF16, tag="exp", name="exF")
                    nc.scalar.activation(out=exF, in_=stF, func=EXP,
                                         scale=SCALE)
                    for h in range(HL):
                        if kc == 0:
                            nc.tensor.ldweights(vaug_sb[:, vidx(b, h, kc), :])
                        nc.tensor.matmul(
                            otF[0:D + 1, h * 512:(h + 1) * 512],
                            lhsT=vaug_sb[:, vidx(b, h, kc), :],
                            rhs=exF[:, h * 512:(h + 1) * 512],
                            start=(kc == 0), stop=(kc == NKC - 1))

                prev = s_pair(0)
                cur = s_pair(1)
                for kc in range(2, NKC):
                    exp_pv(kc - 2, prev)
                    prev, cur = cur, s_pair(kc)
                    if kc % 3 == 1:
                        if pending:
                            pending.pop(0)()
                        elif fillers:
                            fillers.pop(0)()
                exp_pv(NKC - 2, prev)
                exp_pv(NKC - 1, cur)
                while pending:
                    pending.pop(0)()
                normalize_and_a2a(qc, b, otF, qsl)()

        while fillers:
            fillers.pop(0)()

        # ---------------- tail: last two b1 A2A chunks + their proj ------
        gather_a2a(1, 1)
        gather_a2a(1, 2)
        st_mid, _ = A2A_CHUNKS[1][1]
        proj_rows(1, st_mid // CORES, (N - st_mid) // CORES)


_CACHE = {}


def _get_nc():
    if "nc" not in _CACHE:
        _CACHE["nc"] = build_nc()
    return _CACHE["nc"]


def make_in_maps(x, Wq, Wk, Wv, Wproj, bproj):
    bf = ml_dtypes.bfloat16
    x = np.asarray(x, dtype=np.float32).reshape(R, C)
    xT = np.ascontiguousarray(x.T).astype(bf)
    wpT = np.ascontiguousarray(np.asarray(Wproj, np.float32).T).astype(bf)
    bp = np.ascontiguousarray(np.asarray(bproj, np.float32))
    in_maps = []
    for i in range(CORES):
        hs = slice(DL * i, DL * (i + 1))
        in_maps.append({
            "xT": xT,
            "wqT": np.ascontiguousarray(np.asarray(Wq, np.float32)[hs].T).astype(bf),
            "wkT": np.ascontiguousarray(np.asarray(Wk, np.float32)[hs].T).astype(bf),
            "wvT": np.ascontiguousarray(np.asarray(Wv, np.float32)[hs].T).astype(bf),
            "wpT": wpT,
            "bproj": bp,
        })
    return in_maps


def assemble_out(results):
    # core i's rows: per batch b and A2A chunk (st0, nr): global rows
    # b*N + st0 + (nr//CORES)*i, stored at out[b*RB + st0//CORES]
    y = np.zeros((R, C), np.float32)
    for i in range(CORES):
        o = results[i]["out"]
        for b in range(B):
            for st0, nr in A2A_CHUNKS[b]:
                blk = nr // CORES
                g = b * N + st0 + blk * i
                c = b * RB + st0 // CORES
                y[g:g + blk] = o[c:c + blk]
    return y.reshape(B, N, C)


def kernel(x, Wq, Wk, Wv, Wproj, bproj):
    nc = _get_nc()
    in_maps = make_in_maps(x, Wq, Wk, Wv, Wproj, bproj)
    res = run_bass_kernel_spmd(nc, in_maps, core_ids=list(range(CORES)))
    return assemble_out(res.results)
